# revision 14
# baseline (speedup 1.0000x reference)
"""Trainium2 Bass kernel for nn_Attention_34325378629934 (XCA-style channel attention).

Sharding: 8 cores = 4 batches x 2 spatial halves (128 rows each).
Per core:
  1x1 qkv conv as PE matmul (bias via augmented ones-channel, K=193)
  depthwise 3x3: per-tap tensor_scalar (4x mode) + tensor_tensor add (2x) on DVE
    for channel-tiles 0,1,4; GpSimd fp32 tensor_tensor path for tiles 2,3
  q/k head Gram matrices: PE transpose + PSUM-accumulated PE matmuls
  pairwise AllReduce (tiny) of Gram stats between the 2 cores of each batch
  l2-norm row/col scaling + softmax on 24x24 logits per head
  attn @ v via block-diag lhsT (PE), 1x1 proj (PE), bias via ACT
"""
import sys
from contextlib import ExitStack

sys.path.insert(0, "/opt/trn_rl_repo")

import numpy as np
import ml_dtypes

import concourse.bass as bass
import concourse.mybir as mybir
import concourse.tile as tile
from concourse import bacc
from concourse.bass_utils import run_bass_kernel_spmd
from concourse.masks import make_identity

BF16 = ml_dtypes.bfloat16
f32 = mybir.dt.float32
bf16 = mybir.dt.bfloat16

N_CORES = 8
B, C, H, W = 4, 192, 256, 256
C3 = 3 * C
HEADS, HC = 8, 24
RH = 128                 # rows per core
S = RH * W               # 32768
R = 16                   # rows per chunk (DVE tiles)
CHUNKS = RH // R         # 8
KAUG = C + 1             # 193
EPS = 1e-12
AluOp = None  # set below

# channel tiles of qkv output: (offset, size, engine)
TILE_DEFS = [
    (0,   128, "dve"),
    (128, 128, "dve"),
    (256, 128, "gp"),
    (384, 128, "gp"),
    (512, 64,  "dve"),
]
TAPS = [(dy, dx) for dy in range(3) for dx in range(3)]  # center = index 4

_COMPILED = {}


def _emit_dw(nc, inb, acc, dww, dwb, msz, rows, eng, tmpp, tmptag, tmpdt):
    """depthwise accumulate: acc[ch, r, x] = sum_taps w*inb[ch, r+dy, x+dx] + dwb."""
    mult, add = mybir.AluOpType.mult, mybir.AluOpType.add
    nc.vector.tensor_scalar(
        out=acc[:], in0=inb[:, 1:rows + 1, 1:W + 1],
        scalar1=dww[:msz, 4:5], scalar2=dwb[:msz, 0:1], op0=mult, op1=add)
    for tap, (dy, dx) in enumerate(TAPS):
        if tap == 4:
            continue
        shifted = inb[:, dy:dy + rows, dx:dx + W]
        w_ap = dww[:msz, tap:tap + 1]
        tmp = tmpp.tile([msz, rows, W], tmpdt, tag=tmptag)
        if eng == "dve":
            nc.vector.tensor_scalar_mul(tmp[:], shifted, w_ap)
            nc.vector.tensor_tensor(acc[:], acc[:], tmp[:], add)
        else:
            nc.gpsimd.tensor_tensor(tmp[:], shifted,
                                    w_ap.to_broadcast((msz, rows, W)), mult)
            nc.gpsimd.tensor_tensor(acc[:], acc[:], tmp[:], add)


def _build_nc(debug=False):
    nc = bacc.Bacc()

    x_ext = nc.declare_dram_parameter("x", [KAUG, R + 2, CHUNKS, W], bf16, isOutput=False)
    wq_ext = nc.declare_dram_parameter("wq", [5, KAUG, 128], bf16, isOutput=False)
    dww_ext = nc.declare_dram_parameter("dww", [5, 128, 9], f32, isOutput=False)
    dwb_ext = nc.declare_dram_parameter("dwb", [5, 128, 1], f32, isOutput=False)
    wp_ext = nc.declare_dram_parameter("wp", [2, 2, 128, 128], bf16, isOutput=False)
    wpb_ext = nc.declare_dram_parameter("wpb", [2, 128, 1], f32, isOutput=False)
    scl_ext = nc.declare_dram_parameter("scl", [2, 96, 1], f32, isOutput=False)
    y_ext = nc.declare_dram_parameter("y", [C, S], f32, isOutput=True)
    if debug:
        dbg_inb = nc.declare_dram_parameter("dbg_inb", [128, (R + 2) * (W + 2)], bf16, isOutput=True)
        dbg_acc = nc.declare_dram_parameter("dbg_acc", [128, R * W], bf16, isOutput=True)
        dbg_gram = nc.declare_dram_parameter("dbg_gram", [96, 384], f32, isOutput=True)
        dbg_arout = nc.declare_dram_parameter("dbg_arout", [96, 384], f32, isOutput=True)
        dbg_attn = nc.declare_dram_parameter("dbg_attn", [96, 24], bf16, isOutput=True)
        dbg_S = nc.declare_dram_parameter("dbg_S", [96, 24], f32, isOutput=True)
        dbg_qq = nc.declare_dram_parameter("dbg_qq", [96, 1], f32, isOutput=True)
        dbg_kk = nc.declare_dram_parameter("dbg_kk", [96, 24], f32, isOutput=True)
        dbg_av = nc.declare_dram_parameter("dbg_av", [128, 512], bf16, isOutput=True)
        dbg_lA = nc.declare_dram_parameter("dbg_lA", [128, 128], bf16, isOutput=True)
        dbg_ikx = nc.declare_dram_parameter("dbg_ikx", [96, 24], f32, isOutput=True)
        dbg_lgt = nc.declare_dram_parameter("dbg_lgt", [96, 24], f32, isOutput=True)

    with tile.TileContext(nc) as tc, ExitStack() as ctx:
        consts = ctx.enter_context(tc.tile_pool(name="consts", bufs=1))
        xpool = ctx.enter_context(tc.tile_pool(name="xpool", bufs=2))
        inb_b = ctx.enter_context(tc.tile_pool(name="inb_b", bufs=2))
        inb_64 = ctx.enter_context(tc.tile_pool(name="inb_64", bufs=2))
        inb_g = ctx.enter_context(tc.tile_pool(name="inb_g", bufs=2))
        accp = ctx.enter_context(tc.tile_pool(name="accp", bufs=3))
        accp64 = ctx.enter_context(tc.tile_pool(name="accp64", bufs=2))
        accg = ctx.enter_context(tc.tile_pool(name="accg", bufs=2))
        tmpp = ctx.enter_context(tc.tile_pool(name="tmpp", bufs=1))
        v3p = ctx.enter_context(tc.tile_pool(name="v3p", bufs=2))
        qkt = ctx.enter_context(tc.tile_pool(name="qkt", bufs=2))
        smallp = ctx.enter_context(tc.tile_pool(name="smallp", bufs=1))
        iop = ctx.enter_context(tc.tile_pool(name="iop", bufs=2))
        dram = ctx.enter_context(tc.tile_pool(name="dram", bufs=1, space="DRAM"))
        ps_mm = ctx.enter_context(tc.tile_pool(name="ps_mm", bufs=4, space="PSUM"))
        ps_tr = ctx.enter_context(tc.tile_pool(name="ps_tr", bufs=2, space="PSUM"))
        ps_gram = ctx.enter_context(tc.tile_pool(name="ps_gram", bufs=1, space="PSUM"))

        # ---------------- constants ----------------
        ident = consts.tile([128, 128], bf16)
        make_identity(nc, ident)
        wq_sb = []
        for t in range(5):
            k0 = consts.tile([128, 128], bf16, tag=f"wq{t}a")
            k1 = consts.tile([65, 128], bf16, tag=f"wq{t}b")
            nc.sync.dma_start(out=k0[:], in_=wq_ext[t, 0:128, :])
            nc.sync.dma_start(out=k1[:], in_=wq_ext[t, 128:KAUG, :])
            wq_sb.append((k0, k1))
        dww_sb, dwb_sb = [], []
        for t in range(5):
            dwt = consts.tile([128, 9], f32, tag=f"dww{t}")
            nc.sync.dma_start(out=dwt[:], in_=dww_ext[t])
            dww_sb.append(dwt)
            dbt = consts.tile([128, 1], f32, tag=f"dwb{t}")
            nc.sync.dma_start(out=dbt[:], in_=dwb_ext[t])
            dwb_sb.append(dbt)
        wp_sb = [[consts.tile([128, 128], bf16, tag=f"wp{i}{j}", name=f"wp{i}{j}")
                  for j in range(2)] for i in range(2)]
        for i in range(2):
            for j in range(2):
                nc.sync.dma_start(out=wp_sb[i][j][:], in_=wp_ext[i, j])
        wpb_sb = [consts.tile([128, 1], f32, tag=f"wpb{j}", name=f"wpb{j}")
                  for j in range(2)]
        for j in range(2):
            nc.sync.dma_start(out=wpb_sb[j][:], in_=wpb_ext[j])
        scl_sb = [consts.tile([96, 1], f32, tag=f"scl{j}", name=f"scl{j}")
                  for j in range(2)]
        for j in range(2):
            nc.sync.dma_start(out=scl_sb[j][:], in_=scl_ext[j])

        v_spill = dram.tile([C, S], bf16)
        gram_ps = ps_gram.tile([96, 384], f32)

        # ---------------- main loop ----------------
        for c in range(CHUNKS):
            xa = xpool.tile([128, R + 2, W], bf16, tag="xa")
            xb = xpool.tile([65, R + 2, W], bf16, tag="xb")
            nc.sync.dma_start(out=xa[:], in_=x_ext[0:128, :, c, :])
            nc.sync.dma_start(out=xb[:], in_=x_ext[128:KAUG, :, c, :])

            accs = {}

            # ---- DVE tiles (bf16, R=16) ----
            for t, (off, msz, eng) in enumerate(TILE_DEFS):
                if eng != "dve":
                    continue
                pool, tag = (inb_64, "i64") if msz == 64 else (inb_b, "ib")
                inb = pool.tile([msz, R + 2, W + 2], bf16, tag=tag)
                nc.vector.memset(inb[:, :, 0:1], 0.0)
                nc.vector.memset(inb[:, :, W + 1:W + 2], 0.0)
                for n in range((R + 2) // 2):
                    ps = ps_mm.tile([msz, 2, W], f32, tag="ps_mm")
                    nc.tensor.matmul(ps[:], wq_sb[t][0][:, 0:msz],
                                     xa[:, 2 * n:2 * n + 2, :], start=True, stop=False)
                    nc.tensor.matmul(ps[:], wq_sb[t][1][:, 0:msz],
                                     xb[:, 2 * n:2 * n + 2, :], start=False, stop=True)
                    nc.scalar.copy(inb[:, 2 * n:2 * n + 2, 1:W + 1], ps[:])
                if msz == 64:
                    acc = accp64.tile([msz, R, W], bf16, tag="a64")
                else:
                    acc = accp.tile([msz, R, W], bf16, tag="ap")
                _emit_dw(nc, inb, acc, dww_sb[t], dwb_sb[t], msz, R, "dve",
                         tmpp, "t64" if msz == 64 else "tb", bf16)
                accs[t] = acc
                if debug and c == 0 and t == 0:
                    nc.sync.dma_start(out=dbg_inb[:], in_=inb.rearrange("p r w -> p (r w)"))
                    nc.sync.dma_start(out=dbg_acc[:], in_=acc.rearrange("p r w -> p (r w)"))

            # ---- GP tiles (f32, R=8 halves) ----
            acc2_bf = accp.tile([128, R, W], bf16, tag="ap")
            for t, (off, msz, eng) in enumerate(TILE_DEFS):
                if eng != "gp":
                    continue
                for hf in range(2):
                    r8 = R // 2
                    inb = inb_g.tile([msz, r8 + 2, W + 2], f32, tag="ig")
                    nc.vector.memset(inb[:, :, 0:1], 0.0)
                    nc.vector.memset(inb[:, :, W + 1:W + 2], 0.0)
                    for n in range((r8 + 2) // 2):
                        rr = hf * r8 + 2 * n   # row offset within the 18-row x chunk
                        ps = ps_mm.tile([msz, 2, W], f32, tag="ps_mm")
                        nc.tensor.matmul(ps[:], wq_sb[t][0][:, 0:msz],
                                         xa[:, rr:rr + 2, :], start=True, stop=False)
                        nc.tensor.matmul(ps[:], wq_sb[t][1][:, 0:msz],
                                         xb[:, rr:rr + 2, :], start=False, stop=True)
                        nc.scalar.copy(inb[:, 2 * n:2 * n + 2, 1:W + 1], ps[:])
                    acc = accg.tile([msz, r8, W], f32, tag="ag")
                    _emit_dw(nc, inb, acc, dww_sb[t], dwb_sb[t], msz, r8, "gp",
                             tmpp, "tg", f32)
                    if t == 2:
                        nc.vector.tensor_copy(acc2_bf[:, hf * r8:(hf + 1) * r8, :], acc[:])
                    else:
                        vbf = v3p.tile([128, r8 * W], bf16, tag="v3")
                        nc.vector.tensor_copy(vbf[:], acc.rearrange("p r w -> p (r w)"))
                        nc.sync.dma_start(
                            out=v_spill[0:128,
                                        c * R * W + hf * r8 * W:
                                        c * R * W + (hf + 1) * r8 * W],
                            in_=vbf[:])
            # t4 spill
            nc.sync.dma_start(out=v_spill[128:192, c * R * W:(c + 1) * R * W],
                              in_=accs[4].rearrange("p r w -> p (r w)"))

            # ---- transposes + gram ----
            qk_tiles = [accs[0], accs[1], acc2_bf]
            nsb = R * W // 128
            first_c, last_c = (c == 0), (c == CHUNKS - 1)
            for sb in range(nsb):
                trp = ps_tr.tile([128, 384], bf16, tag="tr")
                for t in range(3):
                    blk = qk_tiles[t].rearrange("p r w -> p (r w)")[:, sb * 128:(sb + 1) * 128]
                    nc.tensor.transpose(trp[:, 128 * t:128 * (t + 1)], blk, ident[:])
                # qkT cols grouped per pair: [q_p | k_p] x 4 so gram lhsT is contiguous
                qkT = qkt.tile([128, 384], bf16, tag="qkT")
                qkT4 = qkT.rearrange("p (pr g c) -> p pr g c", pr=4, g=2)
                trq = trp[:, 0:192].rearrange("p (pr c) -> p pr c", pr=4)
                trk = trp[:, 192:384].rearrange("p (pr c) -> p pr c", pr=4)
                nc.scalar.copy(qkT4[:, :, 0, :], trq)
                nc.scalar.copy(qkT4[:, :, 1, :], trk)
                for p in range(4):
                    lhs = qkT[:, 96 * p:96 * (p + 1)]
                    nc.tensor.matmul(gram_ps[:, 96 * p:96 * (p + 1)], lhs, lhs,
                                     start=(first_c and sb == 0),
                                     stop=(last_c and sb == nsb - 1),
                                     skip_group_check=True)

        # ---------------- stats AllReduce ----------------
        gram_sb = smallp.tile([96, 384], f32)
        nc.scalar.copy(gram_sb[:], gram_ps[:])
        ar_in = dram.tile([96, 384], f32)
        ar_out = dram.tile([96, 384], f32)
        nc.sync.dma_start(out=ar_in[:], in_=gram_sb[:])
        nc.gpsimd.collective_compute(
            "AllReduce", mybir.AluOpType.add,
            replica_groups=[[0, 1], [2, 3], [4, 5], [6, 7]],
            ins=[ar_in.opt()], outs=[ar_out.opt()])
        if debug:
            nc.sync.dma_start(out=dbg_gram[:], in_=gram_sb[:])
            nc.sync.dma_start(out=dbg_arout[:], in_=ar_out[:])

        # ---- extract S / diag(qq) / diag(kk) per head-group j (heads 4j..4j+3)
        GW = 384
        attn_bf = []
        for j in range(2):
            st = smallp.tile([96, 24], f32, tag=f"S{j}")
            qt = smallp.tile([96, 1], f32, tag=f"qq{j}")
            kt = smallp.tile([96, 24], f32, tag=f"kk{j}")
            if debug and j == 0:
                dbg_s_pending = (st, qt, kt)
            for l in range(4):
                h = 4 * j + l
                p, d = h // 2, h % 2
                nc.sync.dma_start(
                    out=st[24 * l:24 * l + 24, :],
                    in_=bass.AP(tensor=ar_out.tensor,
                                offset=ar_out.offset + (24 * d) * GW + 96 * p + 48 + 24 * d,
                                ap=[[GW, 24], [1, 24]]))
                nc.sync.dma_start(
                    out=qt[24 * l:24 * l + 24, :],
                    in_=bass.AP(tensor=ar_out.tensor,
                                offset=ar_out.offset + (24 * d) * GW + 96 * p + 24 * d,
                                ap=[[GW + 1, 24], [1, 1]]))
                nc.sync.dma_start(
                    out=kt[24 * l:24 * l + 24, :],
                    in_=bass.AP(tensor=ar_out.tensor,
                                offset=ar_out.offset + (48 + 24 * d) * GW + 96 * p + 48 + 24 * d,
                                ap=[[0, 24], [GW + 1, 24]]))

            if debug and j == 0:
                nc.sync.dma_start(out=dbg_S[:], in_=dbg_s_pending[0][:])
                nc.sync.dma_start(out=dbg_qq[:], in_=dbg_s_pending[1][:])
                nc.sync.dma_start(out=dbg_kk[:], in_=dbg_s_pending[2][:])
            iq = smallp.tile([96, 1], f32, tag=f"iq{j}")
            nc.scalar.sqrt(iq[:], qt[:])
            nc.vector.tensor_scalar_max(iq[:], iq[:], EPS)
            nc.vector.reciprocal(iq[:], iq[:])
            nc.vector.tensor_tensor(iq[:], iq[:], scl_sb[j][:], mybir.AluOpType.mult)
            ik = smallp.tile([96, 24], f32, tag=f"ik{j}")
            nc.scalar.sqrt(ik[:], kt[:])
            nc.vector.tensor_scalar_max(ik[:], ik[:], EPS)
            nc.vector.reciprocal(ik[:], ik[:])

            nc.vector.tensor_scalar_mul(st[:], st[:], iq[:, 0:1])
            nc.vector.tensor_tensor(st[:], st[:], ik[:], mybir.AluOpType.mult)
            if debug and j == 0:
                nc.sync.dma_start(out=dbg_ikx[:], in_=ik[:])
                nc.sync.dma_start(out=dbg_lgt[:], in_=st[:])
            rmax = smallp.tile([96, 1], f32, tag=f"rm{j}")
            nc.vector.reduce_max(rmax[:], st[:], axis=mybir.AxisListType.X)
            nc.vector.tensor_scalar(out=st[:], in0=st[:], scalar1=rmax[:, 0:1],
                                    scalar2=None, op0=mybir.AluOpType.subtract)
            nc.scalar.activation(st[:], st[:], mybir.ActivationFunctionType.Exp)
            rsum = smallp.tile([96, 1], f32, tag=f"rs{j}")
            nc.vector.reduce_sum(rsum[:], st[:], axis=mybir.AxisListType.X)
            nc.vector.reciprocal(rsum[:], rsum[:])
            ab = smallp.tile([96, 24], bf16, tag=f"at{j}")
            nc.vector.tensor_scalar_mul(ab[:], st[:], rsum[:, 0:1])
            attn_bf.append(ab)
            if debug and j == 0:
                nc.sync.dma_start(out=dbg_attn[:], in_=ab[:])

        attnT = smallp.tile([24, 192], bf16)
        for j in range(2):
            trp = ps_tr.tile([128, 384], bf16, tag="tr")
            nc.tensor.transpose(trp[0:24, 0:96], attn_bf[j][:], ident[0:96, 0:96])
            nc.scalar.copy(attnT[:, 96 * j:96 * (j + 1)], trp[0:24, 0:96])

        lA = smallp.tile([128, 128], bf16, tag="lA")
        lB = smallp.tile([64, 128], bf16, tag="lB")
        lC = smallp.tile([128, 64], bf16, tag="lC")
        lD = smallp.tile([64, 64], bf16, tag="lD")
        for tl in (lA, lB, lC, lD):
            nc.vector.memset(tl[:], 0.0)
        for h in range(5):
            nc.sync.dma_start(out=lA[24 * h:24 * h + 24, 24 * h:24 * h + 24],
                              in_=attnT[0:24, 24 * h:24 * h + 24])
        nc.sync.dma_start(out=lA[120:128, 120:128], in_=attnT[0:8, 120:128])
        nc.sync.dma_start(out=lB[0:16, 120:128], in_=attnT[8:24, 120:128])
        nc.sync.dma_start(out=lC[120:128, 0:16], in_=attnT[0:8, 128:144])
        nc.sync.dma_start(out=lD[0:16, 0:16], in_=attnT[8:24, 128:144])
        nc.sync.dma_start(out=lD[16:40, 16:40], in_=attnT[0:24, 144:168])
        nc.sync.dma_start(out=lD[40:64, 40:64], in_=attnT[0:24, 168:192])

        # ---------------- av + proj ----------------
        NU = 512
        for u in range(S // NU):
            sl = slice(u * NU, (u + 1) * NU)
            v0 = iop.tile([128, NU], bf16, tag="v0")
            v1 = iop.tile([64, NU], bf16, tag="v1")
            nc.sync.dma_start(out=v0[:], in_=v_spill[0:128, sl])
            nc.sync.dma_start(out=v1[:], in_=v_spill[128:192, sl])
            pa0 = ps_mm.tile([128, NU], f32, tag="ps_mm")
            pa1 = ps_mm.tile([64, NU], f32, tag="ps_mm")
            nc.tensor.matmul(pa0[:], lA[:], v0[:], start=True, stop=False)
            nc.tensor.matmul(pa0[:], lB[:], v1[:], start=False, stop=True)
            nc.tensor.matmul(pa1[:], lC[:], v0[:], start=True, stop=False)
            nc.tensor.matmul(pa1[:], lD[:], v1[:], start=False, stop=True)
            av0 = iop.tile([128, NU], bf16, tag="av0")
            av1 = iop.tile([64, NU], bf16, tag="av1")
            nc.scalar.copy(av0[:], pa0[:])
            nc.scalar.copy(av1[:], pa1[:])
            if debug and u == 0:
                nc.sync.dma_start(out=dbg_av[:], in_=av0[:])
                nc.sync.dma_start(out=dbg_lA[:], in_=lA[:])
            py0 = ps_mm.tile([128, NU], f32, tag="ps_mm")
            py1 = ps_mm.tile([64, NU], f32, tag="ps_mm")
            nc.tensor.matmul(py0[:], wp_sb[0][0][:], av0[:], start=True, stop=False)
            nc.tensor.matmul(py0[:], wp_sb[1][0][0:64, :], av1[:], start=False, stop=True)
            nc.tensor.matmul(py1[:], wp_sb[0][1][:, 0:64], av0[:], start=True, stop=False)
            nc.tensor.matmul(py1[:], wp_sb[1][1][0:64, 0:64], av1[:], start=False, stop=True)
            y0 = iop.tile([128, NU], f32, tag="y0")
            y1 = iop.tile([64, NU], f32, tag="y1")
            nc.scalar.activation(y0[:], py0[:], mybir.ActivationFunctionType.Identity,
                                 bias=wpb_sb[0][:, 0:1])
            nc.scalar.activation(y1[:], py1[:], mybir.ActivationFunctionType.Identity,
                                 bias=wpb_sb[1][0:64, 0:1])
            nc.sync.dma_start(out=y_ext[0:128, sl], in_=y0[:])
            nc.sync.dma_start(out=y_ext[128:192, sl], in_=y1[:])

    nc.compile()
    return nc


def _host_prep(x, qkv_w, qkv_b, dw_w, dw_b, scale, proj_w, proj_b):
    qkv_w = np.asarray(qkv_w)[:, :, 0, 0].astype(np.float32)
    qkv_b = np.asarray(qkv_b).astype(np.float32)
    dw_w = np.asarray(dw_w)[:, 0].astype(np.float32).reshape(C3, 9)
    dw_b = np.asarray(dw_b).astype(np.float32)
    scale = np.asarray(scale)[0, :, 0, 0].astype(np.float32)
    proj_w = np.asarray(proj_w)[:, :, 0, 0].astype(np.float32)
    proj_b = np.asarray(proj_b).astype(np.float32)
    x = np.asarray(x).astype(np.float32)

    wa = np.concatenate([qkv_w, qkv_b[:, None]], axis=1)
    wq = np.zeros((5, KAUG, 128), np.float32)
    dww = np.zeros((5, 128, 9), np.float32)
    dwb = np.zeros((5, 128, 1), np.float32)
    moff = [0, 128, 256, 384, 512, 576]
    for t in range(5):
        msz = moff[t + 1] - moff[t]
        wq[t, :, 0:msz] = wa[moff[t]:moff[t + 1]].T
        dww[t, 0:msz] = dw_w[moff[t]:moff[t + 1]]
        dwb[t, 0:msz, 0] = dw_b[moff[t]:moff[t + 1]]
    wp = np.zeros((2, 2, 128, 128), np.float32)
    wpb = np.zeros((2, 128, 1), np.float32)
    for i in range(2):
        for j in range(2):
            kb, mb = 128 * i, 128 * j
            ks, ms = min(128, C - kb), min(128, C - mb)
            wp[i, j, 0:ks, 0:ms] = proj_w[mb:mb + ms, kb:kb + ks].T
    wpb[0, 0:128, 0] = proj_b[0:128]
    wpb[1, 0:64, 0] = proj_b[128:192]
    scl = np.repeat(scale, HC).astype(np.float32).reshape(2, 96, 1)

    shared = {
        "wq": wq.astype(BF16), "dww": dww, "dwb": dwb,
        "wp": wp.astype(BF16), "wpb": wpb, "scl": scl,
    }
    in_maps = []
    for core in range(N_CORES):
        b, half = core // 2, core % 2
        r0 = half * RH
        xs = np.zeros((KAUG, RH + 2, W), np.float32)
        lo, hi = r0 - 1, r0 + RH + 1
        slo, shi = max(lo, 0), min(hi, H)
        xs[0:C, slo - lo:shi - lo, :] = x[b, :, slo:shi, :]
        xs[C, slo - lo:shi - lo, :] = 1.0
        xc = np.zeros((KAUG, R + 2, CHUNKS, W), np.float32)
        for c in range(CHUNKS):
            xc[:, :, c, :] = xs[:, c * R:c * R + R + 2, :]
        in_maps.append({"x": xc.astype(BF16), **shared})
    return in_maps


def kernel(**inputs):
    if "nc" not in _COMPILED:
        _COMPILED["nc"] = _build_nc()
    nc = _COMPILED["nc"]
    in_maps = _host_prep(**inputs)
    last_err = None
    for _attempt in range(3):
        try:
            res = run_bass_kernel_spmd(nc, in_maps, list(range(N_CORES)))
            break
        except Exception as e:
            last_err = e
    else:
        raise last_err
    y = np.zeros((B, C, H, W), np.float32)
    for core in range(N_CORES):
        b, half = core // 2, core % 2
        y[b, :, half * RH:half * RH + RH, :] = (
            res.results[core]["y"].reshape(C, RH, W))
    return y
